# revision 13
# baseline (speedup 1.0000x reference)
"""Trainium2 Bass kernel for nn_MultiHeadAttention_15839839388294.

B=2, S=4096, D=768, H=12, HD=64.  Outputs: (out [B,S,D], probs [B,H,S,S]).

Sharding: 8 cores, each handles one batch element (b = core//4) and 3
consecutive heads (h0 = (core%4)*3).  Each core computes its 3 heads'
Q/K/V projections, attention probs (written in full), context, and a
partial output projection.  Host sums the 4 partial outputs per batch
element and adds bo.

All large matmuls run as float32r (FP22 truncated fp32, full PE speed,
fp32 PSUM accumulation).  Softmax skips max subtraction (scores are
tiny: std ~0.31) and folds both the 1/sqrt(HD) scale and the -ln(Z)
normalization into the ScalarE exp activation.
"""

import numpy as np

import concourse.bass as bass
import concourse.tile as tile
from concourse import mybir
from concourse.bass_utils import run_bass_kernel_spmd
from concourse.masks import make_identity

B, S, D, H = 2, 4096, 768, 12
HD = D // H          # 64
HPC = 3              # heads per core
NCORES = 8
SCALE = 1.0 / float(np.sqrt(HD))  # 0.125

F32 = mybir.dt.float32
F32R = mybir.dt.float32r
AF = mybir.ActivationFunctionType

QG = 2048            # q-group width
NQG = S // QG        # 2
NKC = S // 128       # 32 key chunks


def fix_multi_waits(nc):
    """This container's walrus accepts only ONE sync-wait per instruction.

    Hoist extra waits onto preceding same-engine NOPs (engine queues are
    FIFO, so a preceding wait blocks the engine exactly the same way).
    """
    for fn in nc.m.functions:
        for bb in fn.blocks:
            insts = list(bb.instructions)
            out = []
            changed = False
            for inst in insts:
                si = inst.sync_info
                if si is not None and len(si.on_wait) > 1:
                    waits = list(si.on_wait)
                    for i, w in enumerate(waits[:-1]):
                        nop = mybir.InstNoOp(
                            name=f"{inst.name}_hw{i}", engine=inst.engine
                        )
                        nop.sync_info = mybir.SyncInfo(on_wait=[w], on_update=[])
                        out.append(nop)
                    inst.sync_info = mybir.SyncInfo(
                        on_wait=[waits[-1]], on_update=list(si.on_update)
                    )
                    changed = True
                out.append(inst)
            if changed:
                bb.instructions = out


def build_program():
    nc = bass.Bass("TRN2")

    xqT = nc.dram_tensor("xqT", [D, S], F32, kind="ExternalInput")
    xkT = nc.dram_tensor("xkT", [D, S], F32, kind="ExternalInput")
    xvT = nc.dram_tensor("xvT", [D, S], F32, kind="ExternalInput")
    wqT = nc.dram_tensor("wqT", [D, HPC * HD], F32, kind="ExternalInput")
    wkT = nc.dram_tensor("wkT", [D, HPC * HD], F32, kind="ExternalInput")
    wvT = nc.dram_tensor("wvT", [D, HPC * HD], F32, kind="ExternalInput")
    woT = nc.dram_tensor("woT", [HPC * HD, D], F32, kind="ExternalInput")
    bq = nc.dram_tensor("bq", [HPC * HD, 1], F32, kind="ExternalInput")
    bk = nc.dram_tensor("bk", [HPC * HD, 1], F32, kind="ExternalInput")
    bv = nc.dram_tensor("bv", [HPC * HD, 1], F32, kind="ExternalInput")

    probs_out = nc.dram_tensor("probs", [HPC, S, S], F32, kind="ExternalOutput")
    out_part = nc.dram_tensor("out_part", [S, D], F32, kind="ExternalOutput")

    NC6 = D // 128  # 6 contraction chunks of the input-feature dim

    with tile.TileContext(nc) as tc:
        with (
            tc.tile_pool(name="consts", bufs=1) as consts,
            tc.tile_pool(name="qkv", bufs=1) as qkv,
            tc.tile_pool(name="vnat", bufs=1) as vnat,
            tc.tile_pool(name="ps_main", bufs=1, space="PSUM") as psm,
        ):
            # ---------- constants ----------
            ident = consts.tile([128, 128], F32)
            make_identity(nc, ident[:])
            ident_hi = consts.tile([128, HD], F32)
            nc.vector.memset(ident_hi[:], 0.0)
            nc.gpsimd.dma_start(out=ident_hi[64:128, :], in_=ident[0:HD, 0:HD])

            wq_sb = consts.tile([128, NC6, HPC * HD], F32R)
            wk_sb = consts.tile([128, NC6, HPC * HD], F32R)
            wv_sb = consts.tile([128, NC6, HPC * HD], F32R)
            for w_sb, w_dr in ((wq_sb, wqT), (wk_sb, wkT), (wv_sb, wvT)):
                nc.sync.dma_start(
                    out=w_sb[:], in_=w_dr[:].rearrange("(c p) m -> p c m", p=128).bitcast(F32R)
                )
            wo_sb = consts.tile([HD, HPC, D], F32R)
            for p3 in range(HPC):
                nc.sync.dma_start(
                    out=wo_sb[:, p3, :], in_=woT[p3 * HD : (p3 + 1) * HD, :].bitcast(F32R)
                )
            ones64 = consts.tile([1, HD], F32R)
            nc.vector.memset(ones64[:].bitcast(mybir.dt.uint32), 0x3F800000)
            bq01 = consts.tile([128, 1], F32)
            bq2 = consts.tile([64, 1], F32)
            bk01 = consts.tile([128, 1], F32)
            bk2 = consts.tile([64, 1], F32)
            bv01 = consts.tile([128, 1], F32)
            bv2 = consts.tile([64, 1], F32)
            for (t01, t2), dr in (
                ((bq01, bq2), bq),
                ((bk01, bk2), bk),
                ((bv01, bv2), bv),
            ):
                nc.sync.dma_start(out=t01[:], in_=dr[0:128, :])
                nc.sync.dma_start(out=t2[:], in_=dr[128:192, :])

            # projected Q/K in transposed layout [feature, token]
            QT01 = qkv.tile([128, S], F32R)   # heads 0,1 stacked on partitions
            QT2 = qkv.tile([64, S], F32R)     # head 2
            KT01 = qkv.tile([128, S], F32R)
            KT2 = qkv.tile([64, S], F32R)

            # V natural layout per head: [128 keys, kc, 65] (col 64 = ones)
            v_sb = [
                vnat.tile([128, NKC, HD + 1], F32R, tag=f"v{p3}", name=f"v{p3}")
                for p3 in range(HPC)
            ]
            for p3 in range(HPC):
                nc.vector.memset(v_sb[p3][:].bitcast(mybir.dt.uint32), 0x3F800000)

            # ---------- stage 1: projections (+ inline V transpose) ----------
            with (
                tc.tile_pool(name="xt_stage", bufs=2) as xts,
                tc.tile_pool(name="vt_tmp", bufs=2) as vtp,
            ):
                plans = (
                    (xqT, wq_sb, (bq01, bq2), (QT01, QT2), False),
                    (xkT, wk_sb, (bk01, bk2), (KT01, KT2), False),
                    (xvT, wv_sb, (bv01, bv2), (None, None), True),
                )
                NCH = S // 512  # 8 token chunks
                for x_dr, w_sb, (b01, b2), (T01, T2), is_v in plans:
                    x_re = x_dr[:].rearrange("(c p) n -> p c n", p=128)
                    for n in range(NCH):
                        xt = xts.tile([128, NC6, 512], F32R, tag="xt", name="xt")
                        nc.sync.dma_start(
                            out=xt[:], in_=x_re[:, :, n * 512 : (n + 1) * 512].bitcast(F32R)
                        )
                        for gi, (bias, m0, m1) in enumerate(
                            ((b01, 0, 128), (b2, 128, 192))
                        ):
                            mw = m1 - m0
                            ps = psm.tile([mw, 512], F32, tag="st", name="proj", bufs=2)
                            for c in range(NC6):
                                nc.tensor.matmul(
                                    ps[:],
                                    (w_sb[:, c, m0:m1]),
                                    (xt[:, c, :]),
                                    start=(c == 0),
                                    stop=(c == NC6 - 1),
                                )
                            if not is_v:
                                dst = T01 if gi == 0 else T2
                                nc.scalar.activation(
                                    out=dst[:, n * 512 : (n + 1) * 512],
                                    in_=ps[:],
                                    func=AF.Identity,
                                    bias=bias[:],
                                    scale=1.0,
                                )
                            else:
                                # V: strip tile then transpose per head chunk
                                vt = vtp.tile(
                                    [mw, 512], F32, tag=f"vt{gi}", name=f"vt{gi}"
                                )
                                nc.scalar.activation(
                                    out=vt[:],
                                    in_=ps[:],
                                    func=AF.Identity,
                                    bias=bias[:],
                                    scale=1.0,
                                )
                                heads = (0, 1) if gi == 0 else (2,)
                                for p3 in heads:
                                    pb = 64 * (p3 % 2) if gi == 0 else 0
                                    id_ap = (
                                        ident_hi[64:128, :]
                                        if pb
                                        else ident[0:HD, 0:HD]
                                    )
                                    for j in range(4):
                                        kc = n * 4 + j
                                        tp = psm.tile(
                                            [128, HD], F32, tag="ctx", name="vtp",
                                            bufs=1,
                                        )
                                        nc.tensor.transpose(
                                            tp[:],
                                            vt[pb : pb + HD, j * 128 : (j + 1) * 128],
                                            id_ap,
                                        )
                                        nc.vector.tensor_copy(
                                            v_sb[p3][:, kc, 0:HD], tp[:]
                                        )

            # ---------- stage 2: attention + output projection ----------
            def qk_head(p3):
                if p3 < 2:
                    pb = 64 * p3
                    return QT01[pb : pb + HD, :], KT01[pb : pb + HD, :]
                return QT2[:, :], KT2[:, :]

            with (
                tc.tile_pool(name="expt", bufs=2) as expp,
                tc.tile_pool(name="probs", bufs=3) as prp,
                tc.tile_pool(name="ctxs", bufs=1) as ctxp,
                tc.tile_pool(name="small", bufs=1) as smp,
                tc.tile_pool(name="outsb", bufs=2) as osb,
            ):
                for qg in range(NQG):
                    q0 = qg * QG
                    ctxT = []
                    for p3 in range(HPC):
                        qt_h, kt_h = qk_head(p3)
                        # ---- phase A: S^T -> exp -> PV (+ sums row) ----
                        ctx_ps = psm.tile([HD + 1, QG], F32, tag="ctx", name="ctx_ps", bufs=1)
                        for kc in range(NKC):
                            for hh in range(2):
                                c0 = hh * 1024
                                st = psm.tile([128, 1024], F32, tag="st", name="st", bufs=2)
                                for j in range(2):
                                    nc.tensor.matmul(
                                        st[:, j * 512 : (j + 1) * 512],
                                        (kt_h[:, kc * 128 : (kc + 1) * 128]),
                                        (qt_h[
                                                :,
                                                q0 + c0 + j * 512 : q0
                                                + c0
                                                + (j + 1) * 512,
                                            ]
                                        ),
                                        start=True,
                                        stop=True,
                                    )
                                et = expp.tile([128, 1024], F32R, tag="expt", name="et")
                                nc.scalar.activation(
                                    out=et[:], in_=st[:], func=AF.Exp, scale=SCALE
                                )
                                for j in range(2):
                                    nc.tensor.matmul(
                                        ctx_ps[:, c0 + j * 512 : c0 + (j + 1) * 512],
                                        (v_sb[p3][:, kc, :]),
                                        (et[:, j * 512 : (j + 1) * 512]),
                                        start=(kc == 0),
                                        stop=(kc == NKC - 1),
                                        skip_group_check=True,
                                    )
                        # evacuate ctx + sums
                        ct = ctxp.tile(
                            [HD, QG], F32R, tag=f"ctxT{p3}", name=f"ctxT{p3}"
                        )
                        ctxT.append(ct)
                        s_t = smp.tile([1, QG], F32, tag="s_t", name="s_t")
                        nc.vector.tensor_copy(ct[:], ctx_ps[0:HD, :])
                        nc.vector.tensor_copy(s_t[:], ctx_ps[HD : HD + 1, :])

                        # -ln(Z), transposed to per-partition layout
                        nlnzT = smp.tile([1, QG], F32, tag="nlnzT", name="nlnzT")
                        nc.scalar.activation(
                            out=nlnzT[:], in_=s_t[:], func=AF.Ln, scale=1.0
                        )
                        nc.vector.tensor_scalar_mul(nlnzT[:], nlnzT[:], -1.0)
                        nqt = QG // 128  # 16
                        ztp = psm.tile([128, nqt], F32, tag="ctx", name="ztp", bufs=1)
                        for qt in range(nqt):
                            nc.tensor.transpose(
                                ztp[:, qt : qt + 1],
                                nlnzT[:, qt * 128 : (qt + 1) * 128],
                                ident[0:1, 0:1],
                            )
                        nlnz = smp.tile(
                            [128, nqt], F32, tag=f"nlnz{p3}", name=f"nlnz{p3}"
                        )
                        nc.vector.tensor_copy(nlnz[:], ztp[:])

                        # normalize ctx: broadcast 1/Z across partitions via
                        # a rank-1 matmul (ones64^T @ recip), multiply from PSUM
                        rc = smp.tile([1, QG], F32R, tag="rc", name="rc")
                        with nc.allow_low_precision(reason="1/Z at fp22 is plenty"):
                            nc.vector.reciprocal(rc[:], s_t[:])
                        bc_ps = psm.tile([HD, QG], F32, tag="ctx", name="bc_ps", bufs=1)
                        for j in range(QG // 512):
                            nc.tensor.matmul(
                                bc_ps[:, j * 512 : (j + 1) * 512],
                                ones64[:],
                                rc[:, j * 512 : (j + 1) * 512],
                                start=True,
                                stop=True,
                            )
                        nc.vector.tensor_mul(ct[:], ct[:], bc_ps[:])

                        # ---- phase B: natural scores -> normalized probs ----
                        for qt in range(nqt):
                            qrow = q0 + qt * 128
                            for half in range(2):
                                pt = prp.tile(
                                    [128, 2048], F32, tag="probs", name="pt"
                                )
                                for quar in range(2):
                                    sn = psm.tile(
                                        [128, 1024], F32, tag="st", name="sn",
                                        bufs=2,
                                    )
                                    for j in range(2):
                                        k0 = half * 2048 + quar * 1024 + j * 512
                                        nc.tensor.matmul(
                                            sn[:, j * 512 : (j + 1) * 512],
                                            (qt_h[:, qrow : qrow + 128]),
                                            (kt_h[:, k0 : k0 + 512]),
                                            start=True,
                                            stop=True,
                                        )
                                    nc.scalar.activation(
                                        out=pt[:, quar * 1024 : (quar + 1) * 1024],
                                        in_=sn[:],
                                        func=AF.Exp,
                                        bias=nlnz[:, qt : qt + 1],
                                        scale=SCALE,
                                    )
                                nc.sync.dma_start(
                                    out=probs_out[
                                        p3,
                                        qrow : qrow + 128,
                                        half * 2048 : (half + 1) * 2048,
                                    ],
                                    in_=pt[:],
                                )

                    # ---- output projection for this q-group ----
                    for qc2 in range(QG // 256):  # 8 groups of 2 q-chunks
                        ot = osb.tile([128, 2, D], F32, tag="out", name="ot")
                        for sub in range(2):
                            qc0 = qc2 * 256 + sub * 128
                            ps = psm.tile([128, 2, 512], F32, tag="st", name="ops", bufs=2)
                            for p3 in range(HPC):
                                for j in range(2):
                                    nc.tensor.matmul(
                                        ps[:, j, 0:384],
                                        (ctxT[p3][:, qc0 : qc0 + 128]),
                                        (wo_sb[:, p3, j * 384 : (j + 1) * 384]),
                                        start=(p3 == 0),
                                        stop=(p3 == HPC - 1),
                                        skip_group_check=True,
                                    )
                            nc.vector.tensor_copy(
                                ot[:, sub, :].rearrange("p (j n) -> p j n", j=2),
                                ps[:, :, 0:384],
                            )
                        nc.sync.dma_start(
                            out=out_part[
                                q0 + qc2 * 256 : q0 + (qc2 + 1) * 256, :
                            ].rearrange("(c p) n -> p c n", p=128),
                            in_=ot[:],
                        )

    fix_multi_waits(nc)
    return nc


_NC = None


def _get_program():
    global _NC
    if _NC is None:
        _NC = build_program()
    return _NC


def kernel(q_in, k_in, v_in, Wq, bq, Wk, bk, Wv, bv, Wo, bo):
    q_in = np.asarray(q_in, np.float32)
    k_in = np.asarray(k_in, np.float32)
    v_in = np.asarray(v_in, np.float32)
    Wq = np.asarray(Wq, np.float32)
    Wk = np.asarray(Wk, np.float32)
    Wv = np.asarray(Wv, np.float32)
    Wo = np.asarray(Wo, np.float32)
    bq = np.asarray(bq, np.float32)
    bk = np.asarray(bk, np.float32)
    bv = np.asarray(bv, np.float32)
    bo = np.asarray(bo, np.float32)

    xT = {
        "q": [np.ascontiguousarray(q_in[b].T) for b in range(B)],
        "k": [np.ascontiguousarray(k_in[b].T) for b in range(B)],
        "v": [np.ascontiguousarray(v_in[b].T) for b in range(B)],
    }

    in_maps = []
    for c in range(NCORES):
        b = c // 4
        h0 = (c % 4) * HPC
        sl = slice(h0 * HD, (h0 + HPC) * HD)
        in_maps.append(
            {
                "xqT": xT["q"][b],
                "xkT": xT["k"][b],
                "xvT": xT["v"][b],
                "wqT": np.ascontiguousarray(Wq[sl, :].T),
                "wkT": np.ascontiguousarray(Wk[sl, :].T),
                "wvT": np.ascontiguousarray(Wv[sl, :].T),
                "woT": np.ascontiguousarray(Wo[:, sl].T),
                "bq": np.ascontiguousarray(bq[sl].reshape(-1, 1)),
                "bk": np.ascontiguousarray(bk[sl].reshape(-1, 1)),
                "bv": np.ascontiguousarray(bv[sl].reshape(-1, 1)),
            }
        )

    nc = _get_program()
    res = run_bass_kernel_spmd(nc, in_maps, core_ids=list(range(NCORES)))

    probs = np.empty((B, H, S, S), np.float32)
    out = np.zeros((B, S, D), np.float32)
    for c in range(NCORES):
        b = c // 4
        h0 = (c % 4) * HPC
        r = res.results[c]
        probs[b, h0 : h0 + HPC] = r["probs"]
        out[b] += r["out_part"]
    out += bo
    return out, probs


# revision 19
# speedup vs baseline: 1.2759x; 1.2759x over previous
"""Trainium2 Bass kernel for nn_MultiHeadAttention_15839839388294.

B=2, S=4096, D=768, H=12, HD=64.  Outputs: (out [B,S,D], probs [B,H,S,S]).

Sharding: 8 cores, each handles one batch element (b = core//4) and 3
consecutive heads (h0 = (core%4)*3).  Each core computes its 3 heads'
Q/K/V projections, attention probs (written in full), context, and a
partial output projection.  Host sums the 4 partial outputs per batch
element and adds bo.

All large matmuls run as float32r (FP22 truncated fp32, full PE speed,
fp32 PSUM accumulation).  Softmax skips max subtraction (scores are
tiny: std ~0.31) and folds both the 1/sqrt(HD) scale and the -ln(Z)
normalization into the ScalarE exp activation.

The score matmuls contract over only HD=64 partitions, so they are
issued in pairs on disjoint PE row-groups (rows 0-63 / 64-127) which
the 128x128 array executes concurrently: heads 0 and 1 are laid out
stacked on partitions and processed together; head 2's Q^T/K^T are
duplicated on both partition halves so its score matmuls self-pair.
"""

import numpy as np

import concourse.bass as bass
import concourse.tile as tile
from concourse import mybir
from concourse.bass_utils import run_bass_kernel_spmd
from concourse.masks import make_identity

B, S, D, H = 2, 4096, 768, 12
HD = D // H          # 64
HPC = 3              # heads per core
NCORES = 8
SCALE = 1.0 / float(np.sqrt(HD))  # 0.125

F32 = mybir.dt.float32
F32R = mybir.dt.float32r
U32 = mybir.dt.uint32
AF = mybir.ActivationFunctionType
ONE_F32_BITS = 0x3F800000

QG = 1024            # q-group width
NQG = S // QG        # 4
NKC = S // 128       # 32 key chunks


def fix_multi_waits(nc):
    """This container's walrus accepts only ONE sync-wait per instruction.

    Hoist extra waits onto preceding same-engine NOPs (engine queues are
    FIFO, so a preceding wait blocks the engine exactly the same way).
    """
    for fn in nc.m.functions:
        for bb in fn.blocks:
            insts = list(bb.instructions)
            out = []
            changed = False
            for inst in insts:
                si = inst.sync_info
                if si is not None and len(si.on_wait) > 1:
                    waits = list(si.on_wait)
                    for i, w in enumerate(waits[:-1]):
                        nop = mybir.InstNoOp(
                            name=f"{inst.name}_hw{i}", engine=inst.engine
                        )
                        nop.sync_info = mybir.SyncInfo(on_wait=[w], on_update=[])
                        out.append(nop)
                    inst.sync_info = mybir.SyncInfo(
                        on_wait=[waits[-1]], on_update=list(si.on_update)
                    )
                    changed = True
                out.append(inst)
            if changed:
                bb.instructions = out


def build_program():
    nc = bass.Bass("TRN2")

    xqT = nc.dram_tensor("xqT", [D, S], F32, kind="ExternalInput")
    xkT = nc.dram_tensor("xkT", [D, S], F32, kind="ExternalInput")
    xvT = nc.dram_tensor("xvT", [D, S], F32, kind="ExternalInput")
    wqT = nc.dram_tensor("wqT", [D, HPC * HD], F32, kind="ExternalInput")
    wkT = nc.dram_tensor("wkT", [D, HPC * HD], F32, kind="ExternalInput")
    wvT = nc.dram_tensor("wvT", [D, HPC * HD], F32, kind="ExternalInput")
    woT = nc.dram_tensor("woT", [HPC * HD, D], F32, kind="ExternalInput")
    bq = nc.dram_tensor("bq", [HPC * HD, 1], F32, kind="ExternalInput")
    bk = nc.dram_tensor("bk", [HPC * HD, 1], F32, kind="ExternalInput")
    bv = nc.dram_tensor("bv", [HPC * HD, 1], F32, kind="ExternalInput")

    probs_out = nc.dram_tensor("probs", [HPC, S, S], F32, kind="ExternalOutput")
    out_part = nc.dram_tensor("out_part", [S, D], F32, kind="ExternalOutput")

    NC6 = D // 128  # 6 contraction chunks of the input-feature dim
    AOP = mybir.AluOpType

    with tile.TileContext(nc) as tc:
        with (
            tc.tile_pool(name="consts", bufs=1) as consts,
            tc.tile_pool(name="qkv", bufs=1) as qkv,
            tc.tile_pool(name="vnat", bufs=1) as vnat,
            tc.tile_pool(name="ps_main", bufs=1, space="PSUM") as psm,
        ):
            # PSUM tags (8 banks total):
            #   cx0 [65,1024] bufs=1  -> 2 banks
            #   cx1 [65,1024] bufs=1  -> 2 banks
            #   st  [128,1024] bufs=2 -> 4 banks
            def ps_st(name):
                return psm.tile([128, 1024], F32, tag="st", name=name, bufs=2)

            def ps_cx(name, tag="cx0"):
                return psm.tile([HD + 1, QG], F32, tag=tag, name=name, bufs=1)

            # ---------- constants ----------
            ident = consts.tile([128, 128], F32)
            make_identity(nc, ident[:])
            ident_hi = consts.tile([128, HD], F32)
            nc.vector.memset(ident_hi[:], 0.0)
            nc.gpsimd.dma_start(out=ident_hi[64:128, :], in_=ident[0:HD, 0:HD])

            wq_sb = consts.tile([128, NC6, HPC * HD], F32R)
            wk_sb = consts.tile([128, NC6, HPC * HD], F32R)
            wv_sb = consts.tile([128, NC6, HPC * HD], F32R)
            for w_sb, w_dr in ((wq_sb, wqT), (wk_sb, wkT), (wv_sb, wvT)):
                nc.sync.dma_start(
                    out=w_sb[:],
                    in_=w_dr[:].rearrange("(c p) m -> p c m", p=128).bitcast(F32R),
                )
            wo_sb = consts.tile([HD, HPC, D], F32R)
            for p3 in range(HPC):
                nc.sync.dma_start(
                    out=wo_sb[:, p3, :],
                    in_=woT[p3 * HD : (p3 + 1) * HD, :].bitcast(F32R),
                )
            ones64 = consts.tile([1, HD], F32R)
            nc.vector.memset(ones64[:].bitcast(U32), ONE_F32_BITS)

            # biases: heads01 stacked [128,1]; head2 duplicated [128,1]
            # (for the partition-duplicated Q^T/K^T); V head2 plain [64,1]
            bq01 = consts.tile([128, 1], F32)
            bq2d = consts.tile([128, 1], F32)
            bk01 = consts.tile([128, 1], F32)
            bk2d = consts.tile([128, 1], F32)
            bv01 = consts.tile([128, 1], F32)
            bv2 = consts.tile([64, 1], F32)
            for t01, t2, dr, dup in (
                (bq01, bq2d, bq, True),
                (bk01, bk2d, bk, True),
                (bv01, bv2, bv, False),
            ):
                nc.sync.dma_start(out=t01[:], in_=dr[0:128, :])
                nc.sync.dma_start(out=t2[0:64, :], in_=dr[128:192, :])
                if dup:
                    nc.sync.dma_start(out=t2[64:128, :], in_=dr[128:192, :])

            # projected Q/K, transposed layout [feature, token]
            QT01 = qkv.tile([128, S], F32R)  # heads 0,1 stacked on partitions
            QT2 = qkv.tile([128, S], F32R)   # head 2 duplicated on both halves
            KT01 = qkv.tile([128, S], F32R)
            KT2 = qkv.tile([128, S], F32R)

            # V natural layout per head: [128 keys, kc, 65] (col 64 = ones)
            v_sb = [
                vnat.tile([128, NKC, HD + 1], F32R, tag=f"v{p3}", name=f"v{p3}")
                for p3 in range(HPC)
            ]
            for p3 in range(HPC):
                nc.vector.memset(v_sb[p3][:].bitcast(U32), ONE_F32_BITS)

            # ---------- stage 1: projections (+ inline V transpose) ----------
            with (
                tc.tile_pool(name="xt_stage", bufs=2) as xts,
                tc.tile_pool(name="vt_tmp", bufs=2) as vtp,
            ):
                NCH = S // 512  # 8 token chunks
                qk_plans = (
                    (xqT, wq_sb, bq01, bq2d, QT01, QT2),
                    (xkT, wk_sb, bk01, bk2d, KT01, KT2),
                )
                for x_dr, w_sb, b01, b2d, T01, T2 in qk_plans:
                    x_re = x_dr[:].rearrange("(c p) n -> p c n", p=128)
                    for n in range(NCH):
                        nsl = slice(n * 512, (n + 1) * 512)
                        xt = xts.tile([128, NC6, 512], F32R, tag="xt", name="xt")
                        nc.sync.dma_start(out=xt[:], in_=x_re[:, :, nsl].bitcast(F32R))
                        # heads 0,1 (M=128)
                        pA = ps_st("pA")
                        for c in range(NC6):
                            nc.tensor.matmul(
                                pA[:, 0:512],
                                w_sb[:, c, 0:128],
                                xt[:, c, :],
                                start=(c == 0),
                                stop=(c == NC6 - 1),
                            )
                        nc.vector.tensor_scalar_add(T01[:, nsl], pA[:, 0:512], b01[:])
                        # head 2 (M=64); duplicated to the upper partition
                        # half after the loop via SBUF->SBUF DMA
                        pB = ps_st("pB")
                        for c in range(NC6):
                            nc.tensor.matmul(
                                pB[0:64, 0:512],
                                w_sb[:, c, 128:192],
                                xt[:, c, :],
                                start=(c == 0),
                                stop=(c == NC6 - 1),
                            )
                        nc.vector.tensor_scalar_add(
                            T2[0:64, nsl], pB[0:64, 0:512], b2d[0:64, :]
                        )
                    nc.gpsimd.dma_start(out=T2[64:128, :], in_=T2[0:64, :])

                # V projection + transpose to natural layout
                x_re = xvT[:].rearrange("(c p) n -> p c n", p=128)
                for n in range(NCH):
                    xt = xts.tile([128, NC6, 512], F32R, tag="xt", name="xt")
                    nc.sync.dma_start(
                        out=xt[:],
                        in_=x_re[:, :, n * 512 : (n + 1) * 512].bitcast(F32R),
                    )
                    for gi, (bias, m0, m1) in enumerate(
                        ((bv01, 0, 128), (bv2, 128, 192))
                    ):
                        mw = m1 - m0
                        ps = ps_st("pv")
                        for c in range(NC6):
                            nc.tensor.matmul(
                                ps[0:mw, 0:512],
                                wv_sb[:, c, m0:m1],
                                xt[:, c, :],
                                start=(c == 0),
                                stop=(c == NC6 - 1),
                            )
                        vt = vtp.tile([mw, 512], F32, tag=f"vt{gi}", name=f"vt{gi}")
                        nc.vector.tensor_scalar_add(vt[:], ps[0:mw, 0:512], bias[:])
                        heads = (0, 1) if gi == 0 else (2,)
                        for p3 in heads:
                            pb = 64 * (p3 % 2) if gi == 0 else 0
                            id_ap = ident_hi[64:128, :] if pb else ident[0:HD, 0:HD]
                            for j in range(4):
                                kc = n * 4 + j
                                tp = psm.tile(
                                    [128, HD], F32, tag="cx0", name="vtp", bufs=1
                                )
                                nc.tensor.transpose(
                                    tp[:],
                                    vt[pb : pb + HD, j * 128 : (j + 1) * 128],
                                    id_ap,
                                )
                                nc.vector.tensor_copy(v_sb[p3][:, kc, 0:HD], tp[:])

            # ---------- stage 2: attention + output projection ----------
            with (
                tc.tile_pool(name="expt", bufs=3) as expp,
                tc.tile_pool(name="probs", bufs=3) as prp,
                tc.tile_pool(name="ctxs", bufs=1) as ctxp,
                tc.tile_pool(name="small", bufs=1) as smp,
                tc.tile_pool(name="outsb", bufs=2) as osb,
            ):

                def finish_head(p3, cx_ps, cx_tag, q0):
                    """Evacuate ctx+sums, build -lnZ (per-partition) and the
                    1/Z row broadcast, normalize ctx.  Returns (ct, nlnz)."""
                    ct = ctxp.tile([HD, QG], F32R, tag=f"ctxT{p3}", name=f"ctxT{p3}")
                    s_t = smp.tile([1, QG], F32, tag="s_t", name=f"s_t{p3}", bufs=2)
                    nc.vector.tensor_copy(ct[:], cx_ps[0:HD, :])
                    nc.vector.tensor_copy(s_t[:], cx_ps[HD : HD + 1, :])

                    nlnzT = smp.tile([1, QG], F32, tag="lnT", name=f"lnT{p3}", bufs=2)
                    nc.scalar.activation(out=nlnzT[:], in_=s_t[:], func=AF.Ln, scale=1.0)
                    nc.vector.tensor_scalar_mul(nlnzT[:], nlnzT[:], -1.0)
                    nqt = QG // 128  # 8
                    ztp = psm.tile([128, nqt], F32, tag=cx_tag, name="ztp", bufs=1)
                    for qt in range(nqt):
                        nc.tensor.transpose(
                            ztp[:, qt : qt + 1],
                            nlnzT[:, qt * 128 : (qt + 1) * 128],
                            ident[0:1, 0:1],
                        )
                    nlnz = smp.tile([128, nqt], F32, tag=f"nz{p3}", name=f"nz{p3}")
                    nc.vector.tensor_copy(nlnz[:], ztp[:])

                    rc = smp.tile([1, QG], F32R, tag="rc", name=f"rc{p3}", bufs=2)
                    with nc.allow_low_precision(reason="1/Z at fp22 is plenty"):
                        nc.vector.reciprocal(rc[:], s_t[:])
                    bc_ps = psm.tile([HD, QG], F32, tag=cx_tag, name="bc_ps", bufs=1)
                    for j in range(QG // 512):
                        nc.tensor.matmul(
                            bc_ps[:, j * 512 : (j + 1) * 512],
                            ones64[:],
                            rc[:, j * 512 : (j + 1) * 512],
                            start=True,
                            stop=True,
                        )
                    nc.vector.tensor_mul(ct[:], ct[:], bc_ps[:])
                    return ct, nlnz

                def phase_b(p3, qt_t, kt_t, rlo, rhi, nlnz, q0):
                    """Natural scores -> normalized probs for one head.
                    rlo/rhi: partition halves used for the paired fills."""
                    nqt = QG // 128
                    for qt in range(nqt):
                        qrow = q0 + qt * 128
                        for half in range(2):
                            pt = prp.tile([128, 2048], F32, tag="probs", name="pt")
                            for quar in range(2):
                                sn = ps_st("sn")
                                k0 = half * 2048 + quar * 1024
                                nc.tensor.matmul(
                                    sn[:, 0:512],
                                    qt_t[rlo, qrow : qrow + 128],
                                    kt_t[rlo, k0 : k0 + 512],
                                    start=True,
                                    stop=True,
                                )
                                nc.tensor.matmul(
                                    sn[:, 512:1024],
                                    qt_t[rhi, qrow : qrow + 128],
                                    kt_t[rhi, k0 + 512 : k0 + 1024],
                                    start=True,
                                    stop=True,
                                )
                                nc.scalar.activation(
                                    out=pt[:, quar * 1024 : (quar + 1) * 1024],
                                    in_=sn[:],
                                    func=AF.Exp,
                                    bias=nlnz[:, qt : qt + 1],
                                    scale=SCALE,
                                )
                            nc.sync.dma_start(
                                out=probs_out[
                                    p3,
                                    qrow : qrow + 128,
                                    half * 2048 : (half + 1) * 2048,
                                ],
                                in_=pt[:],
                            )

                LO = slice(0, 64)
                HI = slice(64, 128)
                for qg in range(NQG):
                    q0 = qg * QG

                    # ---- phase A, heads 0+1 paired on PE row groups ----
                    cx0 = ps_cx("cx0", "cx0")
                    cx1 = ps_cx("cx1", "cx1")
                    for kc in range(NKC):
                        ksl = slice(kc * 128, (kc + 1) * 128)
                        for qh in range(2):
                            qsl = slice(q0 + qh * 512, q0 + (qh + 1) * 512)
                            osl = slice(qh * 512, (qh + 1) * 512)
                            slab = ps_st("slab")
                            nc.tensor.matmul(
                                slab[:, 0:512], KT01[LO, ksl], QT01[LO, qsl],
                                start=True, stop=True,
                            )
                            nc.tensor.matmul(
                                slab[:, 512:1024], KT01[HI, ksl], QT01[HI, qsl],
                                start=True, stop=True,
                            )
                            et = expp.tile([128, 1024], F32R, tag="expt", name="et")
                            nc.scalar.activation(
                                out=et[:], in_=slab[:], func=AF.Exp, scale=SCALE
                            )
                            nc.tensor.matmul(
                                cx0[:, osl], v_sb[0][:, kc, :], et[:, 0:512],
                                start=(kc == 0), stop=(kc == NKC - 1),
                                skip_group_check=True,
                            )
                            nc.tensor.matmul(
                                cx1[:, osl], v_sb[1][:, kc, :], et[:, 512:1024],
                                start=(kc == 0), stop=(kc == NKC - 1),
                                skip_group_check=True,
                            )
                    ct0, nlnz0 = finish_head(0, cx0, "cx0", q0)
                    ct1, nlnz1 = finish_head(1, cx1, "cx1", q0)

                    # ---- phase A, head 2 self-paired over key-chunk pairs ----
                    cx2 = ps_cx("cx2", "cx1")
                    for kcp in range(NKC // 2):
                        ka = slice(2 * kcp * 128, (2 * kcp + 1) * 128)
                        kb = slice((2 * kcp + 1) * 128, (2 * kcp + 2) * 128)
                        for qh in range(2):
                            qsl = slice(q0 + qh * 512, q0 + (qh + 1) * 512)
                            osl = slice(qh * 512, (qh + 1) * 512)
                            slab = ps_st("slab2")
                            nc.tensor.matmul(
                                slab[:, 0:512], KT2[LO, ka], QT2[LO, qsl],
                                start=True, stop=True,
                            )
                            nc.tensor.matmul(
                                slab[:, 512:1024], KT2[HI, kb], QT2[HI, qsl],
                                start=True, stop=True,
                            )
                            et = expp.tile([128, 1024], F32R, tag="expt", name="et")
                            nc.scalar.activation(
                                out=et[:], in_=slab[:], func=AF.Exp, scale=SCALE
                            )
                            nc.tensor.matmul(
                                cx2[:, osl], v_sb[2][:, 2 * kcp, :], et[:, 0:512],
                                start=(kcp == 0), stop=False,
                                skip_group_check=True,
                            )
                            nc.tensor.matmul(
                                cx2[:, osl], v_sb[2][:, 2 * kcp + 1, :],
                                et[:, 512:1024],
                                start=False, stop=(kcp == NKC // 2 - 1),
                                skip_group_check=True,
                            )
                    ct2, nlnz2 = finish_head(2, cx2, "cx1", q0)
                    ctxT = [ct0, ct1, ct2]

                    # ---- phase B: probs for each head ----
                    phase_b(0, QT01, KT01, LO, LO, nlnz0, q0)
                    phase_b(1, QT01, KT01, HI, HI, nlnz1, q0)
                    phase_b(2, QT2, KT2, LO, HI, nlnz2, q0)

                    # ---- output projection for this q-group ----
                    for qc2 in range(QG // 256):  # 4 groups of 2 q-chunks
                        ot = osb.tile([128, 2, D], F32, tag="out", name="ot")
                        for sub in range(2):
                            qc0 = qc2 * 256 + sub * 128
                            ps = ps_st("ops")
                            for p3 in range(HPC):
                                for j in range(2):
                                    nc.tensor.matmul(
                                        ps[:, j * 512 : j * 512 + 384],
                                        ctxT[p3][:, qc0 : qc0 + 128],
                                        wo_sb[:, p3, j * 384 : (j + 1) * 384],
                                        start=(p3 == 0),
                                        stop=(p3 == HPC - 1),
                                        skip_group_check=True,
                                    )
                            nc.vector.tensor_copy(
                                ot[:, sub, :].rearrange("p (j n) -> p j n", j=2),
                                ps[:].rearrange("p (j n) -> p j n", j=2)[:, :, 0:384],
                            )
                        nc.sync.dma_start(
                            out=out_part[
                                q0 + qc2 * 256 : q0 + (qc2 + 1) * 256, :
                            ].rearrange("(c p) n -> p c n", p=128),
                            in_=ot[:],
                        )

    fix_multi_waits(nc)
    return nc


_NC = None


def _get_program():
    global _NC
    if _NC is None:
        _NC = build_program()
    return _NC


def kernel(q_in, k_in, v_in, Wq, bq, Wk, bk, Wv, bv, Wo, bo):
    q_in = np.asarray(q_in, np.float32)
    k_in = np.asarray(k_in, np.float32)
    v_in = np.asarray(v_in, np.float32)
    Wq = np.asarray(Wq, np.float32)
    Wk = np.asarray(Wk, np.float32)
    Wv = np.asarray(Wv, np.float32)
    Wo = np.asarray(Wo, np.float32)
    bq = np.asarray(bq, np.float32)
    bk = np.asarray(bk, np.float32)
    bv = np.asarray(bv, np.float32)
    bo = np.asarray(bo, np.float32)

    xT = {
        "q": [np.ascontiguousarray(q_in[b].T) for b in range(B)],
        "k": [np.ascontiguousarray(k_in[b].T) for b in range(B)],
        "v": [np.ascontiguousarray(v_in[b].T) for b in range(B)],
    }

    in_maps = []
    for c in range(NCORES):
        b = c // 4
        h0 = (c % 4) * HPC
        sl = slice(h0 * HD, (h0 + HPC) * HD)
        in_maps.append(
            {
                "xqT": xT["q"][b],
                "xkT": xT["k"][b],
                "xvT": xT["v"][b],
                "wqT": np.ascontiguousarray(Wq[sl, :].T),
                "wkT": np.ascontiguousarray(Wk[sl, :].T),
                "wvT": np.ascontiguousarray(Wv[sl, :].T),
                "woT": np.ascontiguousarray(Wo[:, sl].T),
                "bq": np.ascontiguousarray(bq[sl].reshape(-1, 1)),
                "bk": np.ascontiguousarray(bk[sl].reshape(-1, 1)),
                "bv": np.ascontiguousarray(bv[sl].reshape(-1, 1)),
            }
        )

    nc = _get_program()
    res = run_bass_kernel_spmd(nc, in_maps, core_ids=list(range(NCORES)))

    probs = np.empty((B, H, S, S), np.float32)
    out = np.zeros((B, S, D), np.float32)
    for c in range(NCORES):
        b = c // 4
        h0 = (c % 4) * HPC
        r = res.results[c]
        probs[b, h0 : h0 + HPC] = r["probs"]
        out[b] += r["out_part"]
    out += bo
    return out, probs
